# revision 2
# baseline (speedup 1.0000x reference)
"""Expert-parallel DeepseekV2 MoE kernel for 8 Trainium2 NeuronCores, v5.

v5 = v4 + shared-expert compute interleaved into the DMA-starved small
expert slots:
  - Slots are processed big-to-small. Slots 0-1 are PE-bound; slots 2-3 are
    DMA-bound (weight streams outpace their small matmuls), so the shared
    MLP's m1 (gate/up over all tokens) is emitted one k-iteration at a time
    between slot-2/3 matmul groups, and m2 (down) between slot-3 down groups.
    PSUM: m1 holds 6 banks across that region; the streaming ring gets 2.
  - Shared-expert input DMAs ride the SP queue right after slot 0's weights
    (slot 0's compute window hides them; ACT queue keeps only ye outputs).
  - bf16 wire format everywhere (fp32 PSUM accumulate), fp32 outputs.
  - One flat bf16 input blob + one flat f32 output blob per core.
  - 4 expert slots per core, caps = per-rank-group max token load; every
    expert's weights are read exactly once fleet-wide, contiguous host-packed
    partition-major DMAs (gate+up fused in 11 uniform 1.05MB chunks; down in
    4 chunks consumed d-chunk-major).
  - Host: routing, token gather/pack, combine-weight scatter-add, partials.
"""

import math

import numpy as np

import concourse.bass as bass
import concourse.tile as tile
from concourse import bacc, mybir
from concourse.bass_utils import run_bass_kernel_spmd

T, D = 1024, 2048
E, I = 32, 1408
TOPK = 6
N_GROUP, TOPK_GROUP = 8, 3
ROUTED_SCALE = 2.5
SHARED_I = 2 * I

NCORES = 8
NSLOT = E // NCORES        # 4 expert slots per core
ISH = SHARED_I // NCORES   # 352
KT = D // 128              # 16
IT = I // 128              # 11
NWCH = IT                  # 11 weight chunks of 2 gu-tiles (256 cols)
IS_SZ = [128, 128, ISH - 256]

F32 = mybir.dt.float32
BF16 = mybir.dt.bfloat16
SILU = mybir.ActivationFunctionType.Silu
IDENT = mybir.ActivationFunctionType.Identity
NPBF16 = mybir.dt.np(BF16)

_PROGRAM_CACHE = {}


def _in_sections(caps):
    secs = [
        ("xt", (128, KT, T)),
        ("wsg", (128, KT, ISH)),
        ("wsu", (128, KT, ISH)),
        ("wsd", (128, 3, D)),
    ]
    for j in range(NSLOT):
        secs.append((f"xg{j}", (128, KT, caps[j])))
    secs.append(("wgu", (NSLOT, NWCH, 128, KT, 256)))
    secs.append(("wd", (NSLOT, 4, 128, IT, 512)))
    return secs


def _out_sections(caps):
    secs = [("ys", (T // 128, 128, D))]
    for j in range(NSLOT):
        secs.append((f"ye{j}", (math.ceil(caps[j] / 128), 128, D)))
    return secs


def _offsets(secs, align=128):
    offs, n = {}, 0
    for name, shape in secs:
        sz = int(np.prod(shape))
        n = (n + align - 1) // align * align
        offs[name] = (n, shape)
        n += sz
    n = (n + align - 1) // align * align
    return offs, n


def _views(ap, offs):
    out = {}
    for name, (off, shape) in offs.items():
        flat = ap[off:off + int(np.prod(shape))]
        axes = " ".join(f"a{i}" for i in range(len(shape)))
        kw = {f"a{i}": s for i, s in enumerate(shape[:-1])}
        out[name] = flat.rearrange(f"({axes}) -> {axes}", **kw)
    return out


def _build_program(caps):
    caps = tuple(int(c) for c in caps)
    in_offs, n_in = _offsets(_in_sections(caps))
    out_offs, n_out = _offsets(_out_sections(caps))

    nc = bacc.Bacc("TRN2", target_bir_lowering=False, debug=False)
    inp = nc.dram_tensor("inp", [n_in], BF16, kind="ExternalInput").ap()
    outp = nc.dram_tensor("outp", [n_out], F32, kind="ExternalOutput").ap()
    iv = _views(inp, in_offs)
    ov = _views(outp, out_offs)

    xt_d, wsg_d, wsu_d, wsd_d = iv["xt"], iv["wsg"], iv["wsu"], iv["wsd"]
    xg_d = [iv[f"xg{j}"] for j in range(NSLOT)]
    wgu_d, wd_d = iv["wgu"], iv["wd"]
    ys_d = ov["ys"]
    ye_d = [ov[f"ye{j}"] for j in range(NSLOT)]

    with tile.TileContext(nc) as tc:
        with (
            tc.tile_pool(name="sh_in", bufs=1) as shin,
            tc.tile_pool(name="xg_pool", bufs=1) as xgpool,
            tc.tile_pool(name="wstream", bufs=3) as wpool,
            tc.tile_pool(name="wdstream", bufs=2) as wdpool,
            tc.tile_pool(name="ye_out", bufs=4) as yepool,
        ):
            # shared-expert SBUF tiles (inputs DMA'd mid-slot-0; see below)
            xt_sb = shin.tile([128, KT, T], BF16, tag="xt_sb")
            wsg_sb = shin.tile([128, KT, ISH], BF16, tag="wsg_sb")
            wsu_sb = shin.tile([128, KT, ISH], BF16, tag="wsu_sb")
            wsd_sb = shin.tile([128, 3, D], BF16, tag="wsd_sb")
            hsg_sb = None
            hs_sb = None

            # ---- shared-expert work-unit generators (emitted interleaved) --
            def m1_units(m1ps, hsg_sb, hs_sb):
                """Yield after each emitted PE group of shared m1."""
                for w_sb, is_gate in ((wsg_sb, True), (wsu_sb, False)):
                    ps = [m1ps.tile([128, 512], F32, tag="m1ps",
                                    name=f"m1ps{int(is_gate)}_{_i}")
                          for _i in range(6)]
                    for k in range(KT):
                        for jj in range(3):
                            sz = IS_SZ[jj]
                            for tch in range(2):
                                nc.tensor.matmul(
                                    ps[jj * 2 + tch][:sz, :],
                                    w_sb[:, k, jj * 128:jj * 128 + sz],
                                    xt_sb[:, k, tch * 512:(tch + 1) * 512],
                                    start=(k == 0), stop=(k == KT - 1),
                                )
                        yield
                    for jj in range(3):
                        sz = IS_SZ[jj]
                        for tch in range(2):
                            tsl = slice(tch * 512, (tch + 1) * 512)
                            if is_gate:
                                nc.scalar.activation(hsg_sb[:sz, jj, tsl],
                                                     ps[jj * 2 + tch][:sz, :],
                                                     SILU)
                            else:
                                nc.vector.tensor_mul(hs_sb[:sz, jj, tsl],
                                                     ps[jj * 2 + tch][:sz, :],
                                                     hsg_sb[:sz, jj, tsl])
                    yield

            def m2_units(psum, hs_sb):
                """Yield after each (tt, dc) group of shared m2."""
                for tt in range(T // 128):
                    for dc in range(4):
                        py = psum.tile([128, 512], F32, tag="ps", name="pym2")
                        for jj in range(3):
                            sz = IS_SZ[jj]
                            nc.tensor.matmul(
                                py[:],
                                hs_sb[:sz, jj, tt * 128:(tt + 1) * 128],
                                wsd_sb[:sz, jj, dc * 512:(dc + 1) * 512],
                                start=(jj == 0), stop=(jj == 2),
                            )
                        ysb = yepool.tile([128, 512], F32, tag="yet",
                                          name=f"ysb{tt}_{dc}")
                        if dc % 2 == 0:
                            nc.vector.tensor_copy(ysb[:], py[:])
                        else:
                            nc.scalar.activation(ysb[:], py[:], IDENT)
                        nc.sync.dma_start(
                            out=ys_d[tt][:, dc * 512:(dc + 1) * 512],
                            in_=ysb[:])
                        yield

            def emit_slot(j, psum, filler, hpool):
                """Emit expert slot j; pull from filler after each PE group."""
                cap = caps[j]
                ntt = math.ceil(cap / 128)
                tchunks = []
                c0 = 0
                while c0 < cap:
                    tchunks.append((c0, min(512, cap - c0)))
                    c0 += 512

                xg_sb = xgpool.tile([128, KT, cap], BF16, tag=f"xg{j}",
                                    name=f"xg_sb{j}")
                nc.sync.dma_start(out=xg_sb[:], in_=xg_d[j][:])

                hg_sb = hpool.tile([128, IT, cap], BF16, tag=f"hg{j}",
                                   name=f"hg{j}")
                h_sb = hpool.tile([128, IT, cap], BF16, tag=f"h{j}",
                                  name=f"h{j}")
                for ic in range(NWCH):
                    w_sb = wpool.tile([128, KT, 256], BF16, tag="wst",
                                      name=f"wst{j}_{ic}")
                    nc.sync.dma_start(out=w_sb[:], in_=wgu_d[j, ic])
                    for a in range(2):
                        g = ic * 2 + a
                        is_gate = g < IT
                        it = g if is_gate else g - IT
                        for c0, csz in tchunks:
                            pg = psum.tile([128, 512], F32, tag="ps",
                                           name=f"pg{j}_{g}")
                            for k in range(KT):
                                nc.tensor.matmul(
                                    pg[:, :csz],
                                    w_sb[:, k, a * 128:(a + 1) * 128],
                                    xg_sb[:, k, c0:c0 + csz],
                                    start=(k == 0), stop=(k == KT - 1),
                                )
                            if is_gate:
                                nc.scalar.activation(
                                    hg_sb[:, it, c0:c0 + csz],
                                    pg[:, :csz], SILU)
                            else:
                                nc.vector.tensor_mul(
                                    h_sb[:, it, c0:c0 + csz],
                                    pg[:, :csz],
                                    hg_sb[:, it, c0:c0 + csz])
                            next(filler, None)
                    if j == 0:
                        # shared inputs ride SP between slot0's later chunks,
                        # where the weight-stream prefetch lead can absorb
                        # them without starving the gate matmuls.
                        if ic == 5:
                            nc.sync.dma_start(out=xt_sb[:], in_=xt_d[:])
                        elif ic == 8:
                            nc.sync.dma_start(out=wsg_sb[:], in_=wsg_d[:])
                        elif ic == 10:
                            nc.sync.dma_start(out=wsu_sb[:], in_=wsu_d[:])
                            nc.sync.dma_start(out=wsd_sb[:], in_=wsd_d[:])

                # down: ye[t, d] = h^T.T @ wd, d-chunk outer
                for dc in range(4):
                    wd_sb = wdpool.tile([128, IT, 512], BF16, tag="wdst",
                                        name=f"wd_sb{j}_{dc}")
                    nc.sync.dma_start(out=wd_sb[:], in_=wd_d[j, dc])
                    for tt in range(ntt):
                        tsz = min(128, cap - tt * 128)
                        py = psum.tile([128, 512], F32, tag="ps",
                                       name=f"py{j}_{tt}")
                        for i in range(IT):
                            nc.tensor.matmul(
                                py[:tsz, :],
                                h_sb[:, i, tt * 128:tt * 128 + tsz],
                                wd_sb[:, i, :],
                                start=(i == 0), stop=(i == IT - 1),
                            )
                        yet = yepool.tile([128, 512], F32, tag="yet",
                                          name=f"ye{j}_{tt}_{dc}")
                        nc.vector.tensor_copy(yet[:tsz, :], py[:tsz, :])
                        nc.scalar.dma_start(
                            out=ye_d[j][tt, :, dc * 512:(dc + 1) * 512],
                            in_=yet[:])
                        next(filler, None)

            def empty():
                return iter(())

            # slots 0-1: PE-bound, full 8-bank psum ring, no filler
            with tc.tile_pool(name="psumA", bufs=8, space="PSUM") as psA, \
                 tc.tile_pool(name="hbufsA", bufs=1) as hpA:
                emit_slot(0, psA, empty(), hpA)
                emit_slot(1, psA, empty(), hpA)

            # slots 2-3: DMA-bound; shared m1/m2 fill the PE gaps.
            with (
                tc.tile_pool(name="psumM1", bufs=6, space="PSUM") as psM1,
                tc.tile_pool(name="psumS", bufs=2, space="PSUM") as psS,
                tc.tile_pool(name="hbufsB", bufs=1) as hpB,
                tc.tile_pool(name="sh_tmp", bufs=1) as shtmp,
            ):
                hsg_sb = shtmp.tile([128, 3, T], BF16, tag="hsg")
                hs_sb = shtmp.tile([128, 3, T], BF16, tag="hs")
                m1 = m1_units(psM1, hsg_sb, hs_sb)
                emit_slot(2, psS, m1, hpB)
                emit_slot(3, psS, m1, hpB)
                for _ in m1:
                    pass
                m2 = m2_units(psS, hs_sb)
                for _ in m2:
                    pass

    nc.compile()
    return nc


def get_program(caps):
    key = tuple(caps)
    if key not in _PROGRAM_CACHE:
        _PROGRAM_CACHE[key] = _build_program(key)
    return _PROGRAM_CACHE[key]


def _route_numpy(x, gate_w, bias):
    """Mirror reference.py's grouped top-k routing in fp32 numpy."""
    logits = x @ gate_w
    scores = 1.0 / (1.0 + np.exp(-logits))
    sc = scores + bias[None, :]
    g = sc.reshape(-1, N_GROUP, E // N_GROUP)
    group_scores = np.sort(g, axis=-1)[..., -2:].sum(-1)
    gidx = np.argsort(-group_scores, axis=-1, kind="stable")[:, :TOPK_GROUP]
    gmask = np.zeros((x.shape[0], N_GROUP), np.bool_)
    np.put_along_axis(gmask, gidx, True, axis=-1)
    emask = np.repeat(gmask, E // N_GROUP, axis=-1)
    masked = np.where(emask, sc, -np.inf)
    topk_idx = np.argsort(-masked, axis=-1, kind="stable")[:, :TOPK]
    w = np.take_along_axis(scores, topk_idx, axis=-1)
    w = w / (w.sum(-1, keepdims=True) + 1e-20)
    return topk_idx, w


def _plan(topk_idx, topk_w):
    """Token lists per expert; experts ranked by load into NSLOT groups of
    NCORES; caps = per-group max load (rounded up to 8, min 16)."""
    flat_e = topk_idx.ravel()
    flat_t = np.repeat(np.arange(topk_idx.shape[0]), TOPK)
    flat_w = (topk_w * ROUTED_SCALE).ravel().astype(np.float32)
    order = np.argsort(flat_e, kind="stable")
    sorted_t = flat_t[order]
    sorted_w = flat_w[order]
    counts = np.bincount(flat_e, minlength=E)
    offsets = np.concatenate([[0], np.cumsum(counts)])
    toks = [sorted_t[offsets[e]:offsets[e + 1]] for e in range(E)]
    ws = [sorted_w[offsets[e]:offsets[e + 1]] for e in range(E)]

    rank = np.argsort(-counts, kind="stable")  # experts by load desc
    caps = []
    assign = []  # assign[j][c] = expert id
    for j in range(NSLOT):
        grp = rank[j * NCORES:(j + 1) * NCORES]
        cap = max(16, int(math.ceil(counts[grp].max() / 8) * 8))
        caps.append(cap)
        assign.append(list(grp))
    return caps, assign, toks, ws


def _pack_w(w, kt):
    """[kt*128, C] -> [128, kt, C] with row r = a*128+p at [p, a, :]."""
    C = w.shape[1]
    return np.ascontiguousarray(w.reshape(kt, 128, C).transpose(1, 0, 2))


def build_in_maps(inputs):
    x = np.ascontiguousarray(np.asarray(inputs["hidden_states"], np.float32))
    gate_w = np.asarray(inputs["gate_w"], np.float32)
    bias = np.asarray(inputs["e_score_correction_bias"], np.float32)
    w_gate = np.asarray(inputs["w_gate"], np.float32)
    w_up = np.asarray(inputs["w_up"], np.float32)
    w_down = np.asarray(inputs["w_down"], np.float32)
    ws_gate = np.asarray(inputs["ws_gate"], np.float32)
    ws_up = np.asarray(inputs["ws_up"], np.float32)
    ws_down = np.asarray(inputs["ws_down"], np.float32)

    topk_idx, topk_w = _route_numpy(x, gate_w, bias)
    caps, assign, toks, ws = _plan(topk_idx, topk_w)
    in_offs, n_in = _offsets(_in_sections(caps))

    x_bf = x.astype(NPBF16)
    x_bf_t = np.ascontiguousarray(x_bf.T)                      # [D, T]
    xt_np = np.ascontiguousarray(
        x_bf_t.reshape(KT, 128, T).transpose(1, 0, 2))         # [128, KT, T]

    def put(blob, name, arr):
        off, shape = in_offs[name]
        assert tuple(arr.shape) == tuple(shape), (name, arr.shape, shape)
        blob[off:off + arr.size] = arr.ravel()

    ish0 = np.arange(NCORES) * ISH
    in_maps = []
    for c in range(NCORES):
        blob = np.zeros(n_in, NPBF16)
        put(blob, "xt", xt_np)
        put(blob, "wsg",
            _pack_w(ws_gate[:, ish0[c]:ish0[c] + ISH].astype(NPBF16), KT))
        put(blob, "wsu",
            _pack_w(ws_up[:, ish0[c]:ish0[c] + ISH].astype(NPBF16), KT))
        wsd_np = np.zeros((128, 3, D), NPBF16)
        wsd_c = ws_down[ish0[c]:ish0[c] + ISH, :].astype(NPBF16)
        for j in range(3):
            sz = IS_SZ[j]
            wsd_np[:sz, j, :] = wsd_c[j * 128:j * 128 + sz, :]
        put(blob, "wsd", wsd_np)

        wgu_np = np.empty((NSLOT, NWCH, 128, KT, 256), NPBF16)
        wd_np = np.empty((NSLOT, 4, 128, IT, 512), NPBF16)
        for j in range(NSLOT):
            e = assign[j][c]
            gu = np.concatenate([w_gate[e], w_up[e]], axis=1).astype(NPBF16)
            packed = _pack_w(gu, KT)                           # [128, KT, 2816]
            wgu_np[j] = packed.reshape(128, KT, NWCH, 256).transpose(2, 0, 1, 3)
            pd = _pack_w(w_down[e].astype(NPBF16), IT)         # [128, IT, D]
            wd_np[j] = pd.reshape(128, IT, 4, 512).transpose(2, 0, 1, 3)
            cap = caps[j]
            xg = np.zeros((D, cap), NPBF16)
            idx = toks[e]
            if len(idx):
                xg[:, :len(idx)] = x_bf_t[:, idx]
            put(blob, f"xg{j}",
                np.ascontiguousarray(xg.reshape(KT, 128, cap).transpose(1, 0, 2)))
        put(blob, "wgu", wgu_np)
        put(blob, "wd", wd_np)
        in_maps.append({"inp": blob})
    return in_maps, caps, assign, toks, ws


def kernel(**inputs):
    in_maps, caps, assign, toks, ws = build_in_maps(inputs)
    nc = get_program(caps)
    res = run_bass_kernel_spmd(nc, in_maps, core_ids=list(range(NCORES)))
    out_offs, n_out = _offsets(_out_sections(caps))

    routed = np.zeros((T, D), np.float32)
    shared = np.zeros((T, D), np.float32)
    for c in range(NCORES):
        blob = res.results[c]["outp"]
        for j in range(NSLOT):
            e = assign[j][c]
            idx = toks[e]
            if not len(idx):
                continue
            off, shape = out_offs[f"ye{j}"]
            ntt = shape[0]
            y = blob[off:off + ntt * 128 * D].reshape(ntt * 128, D)[:len(idx)]
            routed[idx] += ws[e][:, None] * y.astype(np.float32)
        off, shape = out_offs["ys"]
        shared += blob[off:off + T * D].reshape(T, D)
    return (routed + shared).astype(np.float32)


# revision 3
# speedup vs baseline: 1.4829x; 1.4829x over previous
"""Expert-parallel DeepseekV2 MoE kernel for 8 Trainium2 NeuronCores, v6.

v5 = v4 + shared-expert compute interleaved into the DMA-starved small
expert slots:
  - Slots are processed big-to-small. Slots 0-1 are PE-bound; slots 2-3 are
    DMA-bound (weight streams outpace their small matmuls), so the shared
    MLP's m1 (gate/up over all tokens) is emitted one k-iteration at a time
    between slot-2/3 matmul groups, and m2 (down) between slot-3 down groups.
    PSUM: m1 holds 6 banks across that region; the streaming ring gets 2.
  - Shared-expert input DMAs ride the SP queue right after slot 0's weights
    (slot 0's compute window hides them; ACT queue keeps only ye outputs).
  - bf16 wire format everywhere (fp32 PSUM accumulate), fp32 outputs.
  - One flat bf16 input blob + one flat f32 output blob per core.
  - 4 expert slots per core, caps = per-rank-group max token load; every
    expert's weights are read exactly once fleet-wide, contiguous host-packed
    partition-major DMAs (gate+up fused in 11 uniform 1.05MB chunks; down in
    4 chunks consumed d-chunk-major).
  - Host: routing, token gather/pack, combine-weight scatter-add, partials.
"""

import math

import numpy as np

import concourse.bass as bass
import concourse.tile as tile
from concourse import bacc, mybir
from concourse.bass_utils import run_bass_kernel_spmd

T, D = 1024, 2048
E, I = 32, 1408
TOPK = 6
N_GROUP, TOPK_GROUP = 8, 3
ROUTED_SCALE = 2.5
SHARED_I = 2 * I

NCORES = 8
NSLOT = E // NCORES        # 4 expert slots per core
ISH = SHARED_I // NCORES   # 352
KT = D // 128              # 16
IT = I // 128              # 11
NWCH = IT                  # 11 weight chunks of 2 gu-tiles (256 cols)
IS_SZ = [128, 128, ISH - 256]

F32 = mybir.dt.float32
BF16 = mybir.dt.bfloat16
SILU = mybir.ActivationFunctionType.Silu
IDENT = mybir.ActivationFunctionType.Identity
NPBF16 = mybir.dt.np(BF16)

_PROGRAM_CACHE = {}


def _in_sections(caps):
    secs = [
        ("xt", (128, KT, T)),
        ("wsg", (128, KT, ISH)),
        ("wsu", (128, KT, ISH)),
        ("wsd", (128, 3, D)),
    ]
    for j in range(NSLOT):
        secs.append((f"xg{j}", (128, KT, caps[j])))
    secs.append(("wgu", (NSLOT, NWCH, 128, KT, 256)))
    secs.append(("wd", (NSLOT, 4, 128, IT, 512)))
    return secs


def _out_sections(caps):
    secs = [("ys", (T // 128, 128, D))]
    for j in range(NSLOT):
        secs.append((f"ye{j}", (math.ceil(caps[j] / 128), 128, D)))
    return secs


def _offsets(secs, align=128):
    offs, n = {}, 0
    for name, shape in secs:
        sz = int(np.prod(shape))
        n = (n + align - 1) // align * align
        offs[name] = (n, shape)
        n += sz
    n = (n + align - 1) // align * align
    return offs, n


def _views(ap, offs):
    out = {}
    for name, (off, shape) in offs.items():
        flat = ap[off:off + int(np.prod(shape))]
        axes = " ".join(f"a{i}" for i in range(len(shape)))
        kw = {f"a{i}": s for i, s in enumerate(shape[:-1])}
        out[name] = flat.rearrange(f"({axes}) -> {axes}", **kw)
    return out


def _build_program(caps):
    caps = tuple(int(c) for c in caps)
    in_offs, n_in = _offsets(_in_sections(caps))
    out_offs, n_out = _offsets(_out_sections(caps))

    nc = bacc.Bacc("TRN2", target_bir_lowering=False, debug=False)
    inp = nc.dram_tensor("inp", [n_in], BF16, kind="ExternalInput").ap()
    outp = nc.dram_tensor("outp", [n_out], BF16, kind="ExternalOutput").ap()
    iv = _views(inp, in_offs)
    ov = _views(outp, out_offs)

    xt_d, wsg_d, wsu_d, wsd_d = iv["xt"], iv["wsg"], iv["wsu"], iv["wsd"]
    xg_d = [iv[f"xg{j}"] for j in range(NSLOT)]
    wgu_d, wd_d = iv["wgu"], iv["wd"]
    ys_d = ov["ys"]
    ye_d = [ov[f"ye{j}"] for j in range(NSLOT)]

    with tile.TileContext(nc) as tc:
        with (
            tc.tile_pool(name="sh_in", bufs=1) as shin,
            tc.tile_pool(name="xg_pool", bufs=1) as xgpool,
            tc.tile_pool(name="wstream", bufs=3) as wpool,
            tc.tile_pool(name="wdstream", bufs=2) as wdpool,
            tc.tile_pool(name="ye_out", bufs=4) as yepool,
        ):
            # shared-expert SBUF tiles (inputs DMA'd mid-slot-0; see below)
            xt_sb = shin.tile([128, KT, T], BF16, tag="xt_sb")
            wsg_sb = shin.tile([128, KT, ISH], BF16, tag="wsg_sb")
            wsu_sb = shin.tile([128, KT, ISH], BF16, tag="wsu_sb")
            wsd_sb = shin.tile([128, 3, D], BF16, tag="wsd_sb")
            hsg_sb = None
            hs_sb = None

            # ---- shared-expert work-unit generators (emitted interleaved) --
            def m1_units(m1ps, hsg_sb, hs_sb):
                """Yield after each emitted PE group of shared m1."""
                for w_sb, is_gate in ((wsg_sb, True), (wsu_sb, False)):
                    ps = [m1ps.tile([128, 512], F32, tag="m1ps",
                                    name=f"m1ps{int(is_gate)}_{_i}")
                          for _i in range(6)]
                    for k in range(KT):
                        for jj in range(3):
                            sz = IS_SZ[jj]
                            for tch in range(2):
                                nc.tensor.matmul(
                                    ps[jj * 2 + tch][:sz, :],
                                    w_sb[:, k, jj * 128:jj * 128 + sz],
                                    xt_sb[:, k, tch * 512:(tch + 1) * 512],
                                    start=(k == 0), stop=(k == KT - 1),
                                )
                        yield
                    for jj in range(3):
                        sz = IS_SZ[jj]
                        for tch in range(2):
                            tsl = slice(tch * 512, (tch + 1) * 512)
                            if is_gate:
                                nc.scalar.activation(hsg_sb[:sz, jj, tsl],
                                                     ps[jj * 2 + tch][:sz, :],
                                                     SILU)
                            else:
                                nc.vector.tensor_mul(hs_sb[:sz, jj, tsl],
                                                     ps[jj * 2 + tch][:sz, :],
                                                     hsg_sb[:sz, jj, tsl])
                    yield

            def m2_units(psum, hs_sb):
                """Yield after each (tt, dc) group of shared m2."""
                for tt in range(T // 128):
                    for dc in range(4):
                        py = psum.tile([128, 512], F32, tag="ps", name="pym2")
                        for jj in range(3):
                            sz = IS_SZ[jj]
                            nc.tensor.matmul(
                                py[:],
                                hs_sb[:sz, jj, tt * 128:(tt + 1) * 128],
                                wsd_sb[:sz, jj, dc * 512:(dc + 1) * 512],
                                start=(jj == 0), stop=(jj == 2),
                            )
                        ysb = yepool.tile([128, 512], BF16, tag="yet",
                                          name=f"ysb{tt}_{dc}")
                        if dc % 2 == 0:
                            nc.vector.tensor_copy(ysb[:], py[:])
                        else:
                            nc.scalar.activation(ysb[:], py[:], IDENT)
                        nc.sync.dma_start(
                            out=ys_d[tt][:, dc * 512:(dc + 1) * 512],
                            in_=ysb[:])
                        yield

            def emit_slot(j, psum, filler, hpool):
                """Emit expert slot j; pull from filler after each PE group."""
                cap = caps[j]
                ntt = math.ceil(cap / 128)
                tchunks = []
                c0 = 0
                while c0 < cap:
                    tchunks.append((c0, min(512, cap - c0)))
                    c0 += 512

                xg_sb = xgpool.tile([128, KT, cap], BF16, tag=f"xg{j}",
                                    name=f"xg_sb{j}")
                nc.sync.dma_start(out=xg_sb[:], in_=xg_d[j][:])

                hg_sb = hpool.tile([128, IT, cap], BF16, tag=f"hg{j}",
                                   name=f"hg{j}")
                h_sb = hpool.tile([128, IT, cap], BF16, tag=f"h{j}",
                                  name=f"h{j}")
                for ic in range(NWCH):
                    w_sb = wpool.tile([128, KT, 256], BF16, tag="wst",
                                      name=f"wst{j}_{ic}")
                    nc.sync.dma_start(out=w_sb[:], in_=wgu_d[j, ic])
                    for a in range(2):
                        g = ic * 2 + a
                        is_gate = g < IT
                        it = g if is_gate else g - IT
                        for c0, csz in tchunks:
                            pg = psum.tile([128, 512], F32, tag="ps",
                                           name=f"pg{j}_{g}")
                            for k in range(KT):
                                nc.tensor.matmul(
                                    pg[:, :csz],
                                    w_sb[:, k, a * 128:(a + 1) * 128],
                                    xg_sb[:, k, c0:c0 + csz],
                                    start=(k == 0), stop=(k == KT - 1),
                                )
                            if is_gate:
                                nc.scalar.activation(
                                    hg_sb[:, it, c0:c0 + csz],
                                    pg[:, :csz], SILU)
                            else:
                                nc.vector.tensor_mul(
                                    h_sb[:, it, c0:c0 + csz],
                                    pg[:, :csz],
                                    hg_sb[:, it, c0:c0 + csz])
                            next(filler, None)
                    if j == 0:
                        # shared inputs ride SP between slot0's later chunks,
                        # where the weight-stream prefetch lead can absorb
                        # them without starving the gate matmuls.
                        if ic == 5:
                            nc.sync.dma_start(out=xt_sb[:], in_=xt_d[:])
                        elif ic == 8:
                            nc.sync.dma_start(out=wsg_sb[:], in_=wsg_d[:])
                        elif ic == 10:
                            nc.sync.dma_start(out=wsu_sb[:], in_=wsu_d[:])
                            nc.sync.dma_start(out=wsd_sb[:], in_=wsd_d[:])

                # down: ye[t, d] = h^T.T @ wd, d-chunk outer
                for dc in range(4):
                    wd_sb = wdpool.tile([128, IT, 512], BF16, tag="wdst",
                                        name=f"wd_sb{j}_{dc}")
                    nc.sync.dma_start(out=wd_sb[:], in_=wd_d[j, dc])
                    for tt in range(ntt):
                        tsz = min(128, cap - tt * 128)
                        py = psum.tile([128, 512], F32, tag="ps",
                                       name=f"py{j}_{tt}")
                        for i in range(IT):
                            nc.tensor.matmul(
                                py[:tsz, :],
                                h_sb[:, i, tt * 128:tt * 128 + tsz],
                                wd_sb[:, i, :],
                                start=(i == 0), stop=(i == IT - 1),
                            )
                        yet = yepool.tile([128, 512], BF16, tag="yet",
                                          name=f"ye{j}_{tt}_{dc}")
                        nc.vector.tensor_copy(yet[:tsz, :], py[:tsz, :])
                        nc.scalar.dma_start(
                            out=ye_d[j][tt, :, dc * 512:(dc + 1) * 512],
                            in_=yet[:])
                        next(filler, None)

            def empty():
                return iter(())

            # slots 0-1: PE-bound, full 8-bank psum ring, no filler
            with tc.tile_pool(name="psumA", bufs=8, space="PSUM") as psA, \
                 tc.tile_pool(name="hbufsA", bufs=1) as hpA:
                emit_slot(0, psA, empty(), hpA)
                emit_slot(1, psA, empty(), hpA)

            # slots 2-3: DMA-bound; shared m1/m2 fill the PE gaps.
            with (
                tc.tile_pool(name="psumM1", bufs=6, space="PSUM") as psM1,
                tc.tile_pool(name="psumS", bufs=2, space="PSUM") as psS,
                tc.tile_pool(name="hbufsB", bufs=1) as hpB,
                tc.tile_pool(name="sh_tmp", bufs=1) as shtmp,
            ):
                hsg_sb = shtmp.tile([128, 3, T], BF16, tag="hsg")
                hs_sb = shtmp.tile([128, 3, T], BF16, tag="hs")
                m1 = m1_units(psM1, hsg_sb, hs_sb)
                emit_slot(2, psS, m1, hpB)
                emit_slot(3, psS, m1, hpB)
                for _ in m1:
                    pass
                m2 = m2_units(psS, hs_sb)
                for _ in m2:
                    pass

    nc.compile()
    return nc


def get_program(caps):
    key = tuple(caps)
    if key not in _PROGRAM_CACHE:
        _PROGRAM_CACHE[key] = _build_program(key)
    return _PROGRAM_CACHE[key]


def _route_numpy(x, gate_w, bias):
    """Mirror reference.py's grouped top-k routing in fp32 numpy."""
    logits = x @ gate_w
    scores = 1.0 / (1.0 + np.exp(-logits))
    sc = scores + bias[None, :]
    g = sc.reshape(-1, N_GROUP, E // N_GROUP)
    group_scores = np.sort(g, axis=-1)[..., -2:].sum(-1)
    gidx = np.argsort(-group_scores, axis=-1, kind="stable")[:, :TOPK_GROUP]
    gmask = np.zeros((x.shape[0], N_GROUP), np.bool_)
    np.put_along_axis(gmask, gidx, True, axis=-1)
    emask = np.repeat(gmask, E // N_GROUP, axis=-1)
    masked = np.where(emask, sc, -np.inf)
    topk_idx = np.argsort(-masked, axis=-1, kind="stable")[:, :TOPK]
    w = np.take_along_axis(scores, topk_idx, axis=-1)
    w = w / (w.sum(-1, keepdims=True) + 1e-20)
    return topk_idx, w


def _plan(topk_idx, topk_w):
    """Token lists per expert; experts ranked by load into NSLOT groups of
    NCORES; caps = per-group max load (rounded up to 8, min 16)."""
    flat_e = topk_idx.ravel()
    flat_t = np.repeat(np.arange(topk_idx.shape[0]), TOPK)
    flat_w = (topk_w * ROUTED_SCALE).ravel().astype(np.float32)
    order = np.argsort(flat_e, kind="stable")
    sorted_t = flat_t[order]
    sorted_w = flat_w[order]
    counts = np.bincount(flat_e, minlength=E)
    offsets = np.concatenate([[0], np.cumsum(counts)])
    toks = [sorted_t[offsets[e]:offsets[e + 1]] for e in range(E)]
    ws = [sorted_w[offsets[e]:offsets[e + 1]] for e in range(E)]

    rank = np.argsort(-counts, kind="stable")  # experts by load desc
    caps = []
    assign = []  # assign[j][c] = expert id
    for j in range(NSLOT):
        grp = rank[j * NCORES:(j + 1) * NCORES]
        cap = max(16, int(math.ceil(counts[grp].max() / 8) * 8))
        caps.append(cap)
        assign.append(list(grp))
    return caps, assign, toks, ws


def _pack_w(w, kt):
    """[kt*128, C] -> [128, kt, C] with row r = a*128+p at [p, a, :]."""
    C = w.shape[1]
    return np.ascontiguousarray(w.reshape(kt, 128, C).transpose(1, 0, 2))


def build_in_maps(inputs):
    x = np.ascontiguousarray(np.asarray(inputs["hidden_states"], np.float32))
    gate_w = np.asarray(inputs["gate_w"], np.float32)
    bias = np.asarray(inputs["e_score_correction_bias"], np.float32)
    w_gate = np.asarray(inputs["w_gate"], np.float32)
    w_up = np.asarray(inputs["w_up"], np.float32)
    w_down = np.asarray(inputs["w_down"], np.float32)
    ws_gate = np.asarray(inputs["ws_gate"], np.float32)
    ws_up = np.asarray(inputs["ws_up"], np.float32)
    ws_down = np.asarray(inputs["ws_down"], np.float32)

    topk_idx, topk_w = _route_numpy(x, gate_w, bias)
    caps, assign, toks, ws = _plan(topk_idx, topk_w)
    in_offs, n_in = _offsets(_in_sections(caps))

    x_bf = x.astype(NPBF16)
    x_bf_t = np.ascontiguousarray(x_bf.T)                      # [D, T]
    xt_np = np.ascontiguousarray(
        x_bf_t.reshape(KT, 128, T).transpose(1, 0, 2))         # [128, KT, T]

    def put(blob, name, arr):
        off, shape = in_offs[name]
        assert tuple(arr.shape) == tuple(shape), (name, arr.shape, shape)
        blob[off:off + arr.size] = arr.ravel()

    ish0 = np.arange(NCORES) * ISH
    in_maps = []
    for c in range(NCORES):
        blob = np.zeros(n_in, NPBF16)
        put(blob, "xt", xt_np)
        put(blob, "wsg",
            _pack_w(ws_gate[:, ish0[c]:ish0[c] + ISH].astype(NPBF16), KT))
        put(blob, "wsu",
            _pack_w(ws_up[:, ish0[c]:ish0[c] + ISH].astype(NPBF16), KT))
        wsd_np = np.zeros((128, 3, D), NPBF16)
        wsd_c = ws_down[ish0[c]:ish0[c] + ISH, :].astype(NPBF16)
        for j in range(3):
            sz = IS_SZ[j]
            wsd_np[:sz, j, :] = wsd_c[j * 128:j * 128 + sz, :]
        put(blob, "wsd", wsd_np)

        wgu_np = np.empty((NSLOT, NWCH, 128, KT, 256), NPBF16)
        wd_np = np.empty((NSLOT, 4, 128, IT, 512), NPBF16)
        for j in range(NSLOT):
            e = assign[j][c]
            gu = np.concatenate([w_gate[e], w_up[e]], axis=1).astype(NPBF16)
            packed = _pack_w(gu, KT)                           # [128, KT, 2816]
            wgu_np[j] = packed.reshape(128, KT, NWCH, 256).transpose(2, 0, 1, 3)
            pd = _pack_w(w_down[e].astype(NPBF16), IT)         # [128, IT, D]
            wd_np[j] = pd.reshape(128, IT, 4, 512).transpose(2, 0, 1, 3)
            cap = caps[j]
            xg = np.zeros((D, cap), NPBF16)
            idx = toks[e]
            if len(idx):
                xg[:, :len(idx)] = x_bf_t[:, idx]
            put(blob, f"xg{j}",
                np.ascontiguousarray(xg.reshape(KT, 128, cap).transpose(1, 0, 2)))
        put(blob, "wgu", wgu_np)
        put(blob, "wd", wd_np)
        in_maps.append({"inp": blob})
    return in_maps, caps, assign, toks, ws


def kernel(**inputs):
    in_maps, caps, assign, toks, ws = build_in_maps(inputs)
    nc = get_program(caps)
    res = run_bass_kernel_spmd(nc, in_maps, core_ids=list(range(NCORES)))
    out_offs, n_out = _offsets(_out_sections(caps))

    routed = np.zeros((T, D), np.float32)
    shared = np.zeros((T, D), np.float32)
    for c in range(NCORES):
        blob = res.results[c]["outp"].astype(np.float32)
        for j in range(NSLOT):
            e = assign[j][c]
            idx = toks[e]
            if not len(idx):
                continue
            off, shape = out_offs[f"ye{j}"]
            ntt = shape[0]
            y = blob[off:off + ntt * 128 * D].reshape(ntt * 128, D)[:len(idx)]
            routed[idx] += ws[e][:, None] * y.astype(np.float32)
        off, shape = out_offs["ys"]
        shared += blob[off:off + T * D].reshape(T, D)
    return (routed + shared).astype(np.float32)
